# revision 6
# baseline (speedup 1.0000x reference)
"""ConvNeXt MLP + parallel MoE-LoRA fused Trainium2 kernel.

Math (per token x in R^C):
  orig = gelu(x @ w1 + b1) @ w2 + b2                      (C=768, HID=3072)
  wt[e] = sum_k topk_probs[k] * [topk_idx[k] == e]        (E=8, K=2)
  down  = gelu(x @ w_down_all)                            (w_down_all: [C, E*R=128])
  moe   = (down * wte) @ w_up_all                         (wte: per-(e,r) token weight)
  out   = orig + moe

Strategy: data-parallel over the N=12544 tokens across 8 NeuronCores
(1568 tokens/core); dense-over-experts MoE (E*R = 128 = one partition dim,
so the whole MoE adds just two extra 128-wide matmul passes; only ~4%
extra FLOPs vs routed dispatch). All activations live in [feature, token]
layout so both MLP matmuls keep the contraction on partitions with zero
transposes; the expert combine weights are built on-device from
topk_idx/topk_probs with fused DVE compare-multiply ops and one PE
transpose per 128 tokens. The MoE up-projection accumulates directly into
the second MLP matmul's PSUM tile, so orig + moe is free.

Matmuls run in float32r (fp32 streamed at 1 row/cycle, ~12 mantissa bits:
~4x fp32 throughput at ~1.5e-4 matmul rel-err). Weights are streamed from
HBM in two passes (token-tile groups of 2) so SBUF holds only 2 token
tiles of hidden activations.
"""

from contextlib import ExitStack

import numpy as np

import concourse.mybir as mybir
import concourse.tile as tile
from concourse import bacc
from concourse.bass_utils import run_bass_kernel_spmd
from concourse.masks import make_identity

F32 = mybir.dt.float32
F32R = mybir.dt.float32r
ACT = mybir.ActivationFunctionType
ALU = mybir.AluOpType

B, H, W, C = 64, 14, 14, 768
HID = 4 * C
E, TOPK, R = 8, 2, 16
ER = E * R              # 128
N = B * H * W           # 12544
NCORES = 8
NL = N // NCORES        # 1568 tokens per core
P = 128
KC = C // P             # 6 contraction chunks for C
MH = HID // P           # 24 hid chunks
MC = C // P             # 6 output chunks
TT = 392                # token tile (moving dim; <=512 and 1568 = 4*392)
NT = NL // TT           # 4 token tiles per core
G = 2                   # token tiles per weight-streaming group
NWT = (NL + P - 1) // P  # 13 token chunks for wte build


def _build(reps=1):
    nc = bacc.Bacc("TRN2", target_bir_lowering=False, debug=False)

    xT = nc.dram_tensor("xT", [C, NL], F32R, kind="ExternalInput")
    w1 = nc.dram_tensor("w1", [C, HID], F32R, kind="ExternalInput")
    b1 = nc.dram_tensor("b1", [HID], F32, kind="ExternalInput")
    w2 = nc.dram_tensor("w2", [HID, C], F32R, kind="ExternalInput")
    b2 = nc.dram_tensor("b2", [C], F32, kind="ExternalInput")
    wd = nc.dram_tensor("wd", [C, ER], F32R, kind="ExternalInput")
    wu = nc.dram_tensor("wu", [ER, C], F32R, kind="ExternalInput")
    idxp = nc.dram_tensor("idxp", [NL, 4], F32, kind="ExternalInput")
    t_er = nc.dram_tensor("t_er", [P, P], F32, kind="ExternalInput")
    outT = nc.dram_tensor("outT", [C, NL], F32, kind="ExternalOutput")

    with tile.TileContext(nc) as tc, ExitStack() as ctx:
        cons = ctx.enter_context(tc.tile_pool(name="cons", bufs=1))
        small = ctx.enter_context(tc.tile_pool(name="small", bufs=4))
        xt_pool = ctx.enter_context(tc.tile_pool(name="xt", bufs=4))
        h_pool = ctx.enter_context(tc.tile_pool(name="h", bufs=2))
        dsc_pool = ctx.enter_context(tc.tile_pool(name="dsc", bufs=2))
        w1_pool = ctx.enter_context(tc.tile_pool(name="w1", bufs=4))
        w2_pool = ctx.enter_context(tc.tile_pool(name="w2", bufs=2))
        out_pool = ctx.enter_context(tc.tile_pool(name="out", bufs=4))
        psum_mm = ctx.enter_context(tc.tile_pool(name="psmm", bufs=6, space="PSUM"))
        psum_tr = ctx.enter_context(tc.tile_pool(name="pstr", bufs=2, space="PSUM"))

        if reps > 1:
            # timing-only variant: run the whole body `reps` times so the
            # per-iteration HW time can be extracted from wall-clock deltas
            ctx.enter_context(tc.For_i(
                0, reps, 1,
                hint_engines=(mybir.EngineType.PE, mybir.EngineType.Activation,
                              mybir.EngineType.DVE, mybir.EngineType.SP,
                              mybir.EngineType.Pool)))

        # --- resident constants ---
        wu_sb = cons.tile([P, C], F32R)
        nc.sync.dma_start(wu_sb[:], wu[:, :])
        wd_sb = cons.tile([P, KC, P], F32R)
        nc.sync.dma_start(wd_sb[:], wd[:, :].rearrange("(k p) m -> p k m", p=P))
        b1_sb = cons.tile([P, MH], F32)
        nc.sync.dma_start(b1_sb[:], b1[:].rearrange("(m p) -> p m", p=P))
        b2_sb = cons.tile([P, MC], F32)
        nc.sync.dma_start(b2_sb[:], b2[:].rearrange("(m p) -> p m", p=P))
        ter_sb = cons.tile([P, P], F32)
        nc.sync.dma_start(ter_sb[:], t_er[:, :])
        ident = cons.tile([P, P], F32)
        make_identity(nc, ident[:])

        # --- expert combine weights wte[er, tok] for the whole shard ---
        wte_sb = cons.tile([P, NL], F32)
        for tch in range(NWT):
            off = tch * P
            pt = min(P, NL - off)
            ip = small.tile([P, 4], F32, tag="ip")
            nc.sync.dma_start(ip[:pt], idxp[off:off + pt, :])
            wa = small.tile([P, P], F32, tag="wa")
            nc.vector.tensor_scalar(wa[:pt], ter_sb[:pt], ip[:pt, 0:1], ip[:pt, 2:3],
                                    ALU.is_equal, ALU.mult)
            wb = small.tile([P, P], F32, tag="wb")
            nc.vector.tensor_scalar(wb[:pt], ter_sb[:pt], ip[:pt, 1:2], ip[:pt, 3:4],
                                    ALU.is_equal, ALU.mult)
            nc.vector.tensor_add(wa[:pt], wa[:pt], wb[:pt])
            pst = psum_tr.tile([P, P], F32)
            nc.tensor.transpose(pst[:, :pt], wa[:pt, :], ident[:pt, :pt])
            nc.scalar.activation(wte_sb[:, off:off + pt], pst[:, :pt], ACT.Copy)

        # --- main pipeline over token-tile groups ---
        for g in range(NT // G):
            tiles = range(g * G, (g + 1) * G)
            xts = {}
            hs = {}
            dscs = {}
            for t in tiles:
                xts[t] = xt_pool.tile([P, KC, TT], F32R, tag="xt", name=f"xt{t}")
                nc.sync.dma_start(
                    xts[t][:],
                    xT[:, t * TT:(t + 1) * TT].rearrange("(k p) n -> p k n", p=P))
                hs[t] = h_pool.tile([P, MH, TT], F32R, tag="h", name=f"h{t}")
                dscs[t] = dsc_pool.tile([P, TT], F32R, tag="dsc", name=f"dsc{t}")

            # phase A: h = gelu(x @ w1 + b1); dsc = gelu(x @ wd) * wte
            for m in range(MH + 1):
                if m < MH:
                    wk = w1_pool.tile([P, KC, P], F32R, tag="w1")
                    nc.sync.dma_start(
                        wk[:],
                        w1[:, m * P:(m + 1) * P].rearrange("(k p) m -> p k m", p=P))
                else:
                    wk = wd_sb
                for t in tiles:
                    ps = psum_mm.tile([P, TT], F32)
                    for k in range(KC):
                        nc.tensor.matmul(ps[:], wk[:, k, :], xts[t][:, k, :],
                                         start=(k == 0), stop=(k == KC - 1))
                    if m < MH:
                        nc.scalar.activation(hs[t][:, m, :], ps[:], ACT.Gelu,
                                             bias=b1_sb[:, m:m + 1])
                    else:
                        gd = small.tile([P, TT], F32, tag="gd")
                        nc.scalar.activation(gd[:], ps[:], ACT.Gelu)
                        nc.vector.tensor_mul(dscs[t][:], gd[:],
                                             wte_sb[:, t * TT:(t + 1) * TT])

            # phase B: out = h @ w2 + b2 + dsc @ wu
            for m2 in range(MC):
                w2k = w2_pool.tile([P, MH, P], F32R, tag="w2")
                nc.sync.dma_start(
                    w2k[:],
                    w2[:, m2 * P:(m2 + 1) * P].rearrange("(k p) m -> p k m", p=P))
                for t in tiles:
                    ps = psum_mm.tile([P, TT], F32)
                    for k in range(MH):
                        nc.tensor.matmul(ps[:], w2k[:, k, :], hs[t][:, k, :],
                                         start=(k == 0), stop=False)
                    nc.tensor.matmul(ps[:], wu_sb[:, m2 * P:(m2 + 1) * P],
                                     dscs[t][:], start=False, stop=True)
                    ob = out_pool.tile([P, TT], F32, tag="ob")
                    nc.scalar.activation(ob[:], ps[:], ACT.Identity,
                                         bias=b2_sb[:, m2:m2 + 1])
                    nc.sync.dma_start(
                        outT[m2 * P:(m2 + 1) * P, t * TT:(t + 1) * TT], ob[:])

    nc.compile()
    return nc


_NC = None


def _get_nc():
    global _NC
    if _NC is None:
        _NC = _build()
    return _NC


def prepare_in_maps(x, topk_probs, topk_idx, w1, b1, w2, b2, w_down, w_up):
    x = np.asarray(x, dtype=np.float32)
    topk_probs = np.asarray(topk_probs, dtype=np.float32)
    topk_idx = np.asarray(topk_idx)
    w1 = np.ascontiguousarray(np.asarray(w1, dtype=np.float32))
    b1 = np.ascontiguousarray(np.asarray(b1, dtype=np.float32))
    w2 = np.ascontiguousarray(np.asarray(w2, dtype=np.float32))
    b2 = np.ascontiguousarray(np.asarray(b2, dtype=np.float32))
    w_down = np.asarray(w_down, dtype=np.float32)
    w_up = np.asarray(w_up, dtype=np.float32)

    xf = x.reshape(N, C)
    wd = np.ascontiguousarray(w_down.transpose(1, 0, 2).reshape(C, ER))
    wu = np.ascontiguousarray(w_up.reshape(ER, C))
    t_er = np.ascontiguousarray(
        np.broadcast_to((np.arange(P) // R).astype(np.float32), (P, P)))

    in_maps = []
    for i in range(NCORES):
        sl = slice(i * NL, (i + 1) * NL)
        in_maps.append({
            "xT": np.ascontiguousarray(xf[sl].T),
            "w1": w1, "b1": b1, "w2": w2, "b2": b2,
            "wd": wd, "wu": wu,
            "idxp": np.ascontiguousarray(np.concatenate(
                [topk_idx[sl].astype(np.float32),
                 topk_probs[sl].astype(np.float32)], axis=1)),
            "t_er": t_er,
        })
    return in_maps


def gather_out(results):
    out = np.concatenate([results[i]["outT"].T for i in range(NCORES)], axis=0)
    return np.ascontiguousarray(out.reshape(B, H, W, C))


def kernel(x, topk_probs, topk_idx, w1, b1, w2, b2, w_down, w_up):
    in_maps = prepare_in_maps(x, topk_probs, topk_idx, w1, b1, w2, b2,
                              w_down, w_up)
    res = run_bass_kernel_spmd(_get_nc(), in_maps, core_ids=list(range(NCORES)))
    return gather_out(res.results)


# revision 9
# speedup vs baseline: 1.5564x; 1.5564x over previous
"""ConvNeXt MLP + parallel MoE-LoRA fused Trainium2 kernel.

Math (per token x in R^C):
  orig = gelu(x @ w1 + b1) @ w2 + b2                      (C=768, HID=3072)
  wt[e] = sum_k topk_probs[k] * [topk_idx[k] == e]        (E=8, K=2)
  down  = gelu(x @ w_down_all)                            (w_down_all: [C, E*R=128])
  moe   = (down * wte) @ w_up_all                         (wte: per-(e,r) token weight)
  out   = orig + moe

Strategy: data-parallel over the N=12544 tokens across 8 NeuronCores
(1568 tokens/core); dense-over-experts MoE (E*R = 128 = one partition dim,
so the whole MoE adds just two extra 128-wide matmul passes; only ~4%
extra FLOPs vs routed dispatch). All activations live in [feature, token]
layout so both MLP matmuls keep the contraction on partitions with zero
transposes; the expert combine weights are built on-device from
topk_idx/topk_probs with fused DVE compare-multiply ops and one PE
transpose per 128 tokens. The MoE up-projection accumulates directly into
the second MLP matmul's PSUM tile, so orig + moe is free.

Matmuls run in float32r (fp32 streamed at 1 row/cycle, ~12 mantissa bits:
~4x fp32 throughput at ~1.5e-4 matmul rel-err). Weights are streamed from
HBM in two passes (token-tile groups of 2) so SBUF holds only 2 token
tiles of hidden activations.
"""

from contextlib import ExitStack

import numpy as np

import concourse.mybir as mybir
import concourse.tile as tile
from concourse import bacc
from concourse.bass_utils import run_bass_kernel_spmd
from concourse.masks import make_identity

F32 = mybir.dt.float32
F32R = mybir.dt.float32r
ACT = mybir.ActivationFunctionType
ALU = mybir.AluOpType

B, H, W, C = 64, 14, 14, 768
HID = 4 * C
E, TOPK, R = 8, 2, 16
ER = E * R              # 128
N = B * H * W           # 12544
NCORES = 8
NL = N // NCORES        # 1568 tokens per core
P = 128
KC = C // P             # 6 contraction chunks for C
MH = HID // P           # 24 hid chunks
MC = C // P             # 6 output chunks
TT = 392                # token tile (moving dim; <=512 and 1568 = 4*392)
NT = NL // TT           # 4 token tiles per core
G = 2                   # token tiles per weight-streaming group
NWT = (NL + P - 1) // P  # 13 token chunks for wte build


def _build(reps=1):
    nc = bacc.Bacc("TRN2", target_bir_lowering=False, debug=False)

    xT = nc.dram_tensor("xT", [C, NL], F32R, kind="ExternalInput")
    w1 = nc.dram_tensor("w1", [C, HID], F32R, kind="ExternalInput")
    b1 = nc.dram_tensor("b1", [HID], F32, kind="ExternalInput")
    w2 = nc.dram_tensor("w2", [HID, C], F32R, kind="ExternalInput")
    b2 = nc.dram_tensor("b2", [C], F32, kind="ExternalInput")
    wd = nc.dram_tensor("wd", [C, ER], F32R, kind="ExternalInput")
    wu = nc.dram_tensor("wu", [ER, C], F32R, kind="ExternalInput")
    idxp = nc.dram_tensor("idxp", [NL, 4], F32, kind="ExternalInput")
    t_er = nc.dram_tensor("t_er", [P, P], F32, kind="ExternalInput")
    outT = nc.dram_tensor("outT", [C, NL], F32, kind="ExternalOutput")

    with tile.TileContext(nc) as tc, ExitStack() as ctx:
        cons = ctx.enter_context(tc.tile_pool(name="cons", bufs=1))
        small = ctx.enter_context(tc.tile_pool(name="small", bufs=4))
        xt_pool = ctx.enter_context(tc.tile_pool(name="xt", bufs=4))
        h_pool = ctx.enter_context(tc.tile_pool(name="h", bufs=2))
        dsc_pool = ctx.enter_context(tc.tile_pool(name="dsc", bufs=2))
        w1_pool = ctx.enter_context(tc.tile_pool(name="w1", bufs=8))
        w2_pool = ctx.enter_context(tc.tile_pool(name="w2", bufs=3))
        out_pool = ctx.enter_context(tc.tile_pool(name="out", bufs=4))
        psum_mm = ctx.enter_context(tc.tile_pool(name="psmm", bufs=6, space="PSUM"))
        psum_tr = ctx.enter_context(tc.tile_pool(name="pstr", bufs=2, space="PSUM"))

        if reps > 1:
            # timing-only variant: run the whole body `reps` times so the
            # per-iteration HW time can be extracted from wall-clock deltas
            ctx.enter_context(tc.For_i(
                0, reps, 1,
                hint_engines=(mybir.EngineType.PE, mybir.EngineType.Activation,
                              mybir.EngineType.DVE, mybir.EngineType.SP,
                              mybir.EngineType.Pool)))

        # --- resident constants ---
        wu_sb = cons.tile([P, C], F32R)
        nc.sync.dma_start(wu_sb[:], wu[:, :])
        wd_sb = cons.tile([P, KC, P], F32R)
        nc.sync.dma_start(wd_sb[:], wd[:, :].rearrange("(k p) m -> p k m", p=P))
        b1_sb = cons.tile([P, MH], F32)
        nc.sync.dma_start(b1_sb[:], b1[:].rearrange("(m p) -> p m", p=P))
        b2_sb = cons.tile([P, MC], F32)
        nc.sync.dma_start(b2_sb[:], b2[:].rearrange("(m p) -> p m", p=P))
        ter_sb = cons.tile([P, P], F32)
        nc.sync.dma_start(ter_sb[:], t_er[:, :])
        ident = cons.tile([P, P], F32)
        make_identity(nc, ident[:])

        # --- expert combine weights wte[er, tok] for the whole shard ---
        wte_sb = cons.tile([P, NL], F32)
        for tch in range(NWT):
            off = tch * P
            pt = min(P, NL - off)
            ip = small.tile([P, 4], F32, tag="ip")
            nc.sync.dma_start(ip[:pt], idxp[off:off + pt, :])
            wa = small.tile([P, P], F32, tag="wa")
            nc.vector.tensor_scalar(wa[:pt], ter_sb[:pt], ip[:pt, 0:1], ip[:pt, 2:3],
                                    ALU.is_equal, ALU.mult)
            wb = small.tile([P, P], F32, tag="wb")
            nc.vector.tensor_scalar(wb[:pt], ter_sb[:pt], ip[:pt, 1:2], ip[:pt, 3:4],
                                    ALU.is_equal, ALU.mult)
            nc.vector.tensor_add(wa[:pt], wa[:pt], wb[:pt])
            pst = psum_tr.tile([P, P], F32)
            nc.tensor.transpose(pst[:, :pt], wa[:pt, :], ident[:pt, :pt])
            nc.scalar.activation(wte_sb[:, off:off + pt], pst[:, :pt], ACT.Copy)

        # --- main pipeline over token-tile groups ---
        for g in range(NT // G):
            tiles = range(g * G, (g + 1) * G)
            xts = {}
            hs = {}
            dscs = {}
            for t in tiles:
                xts[t] = xt_pool.tile([P, KC, TT], F32R, tag="xt", name=f"xt{t}")
                nc.sync.dma_start(
                    xts[t][:],
                    xT[:, t * TT:(t + 1) * TT].rearrange("(k p) n -> p k n", p=P))
                hs[t] = h_pool.tile([P, MH, TT], F32R, tag="h", name=f"h{t}")
                dscs[t] = dsc_pool.tile([P, TT], F32R, tag="dsc", name=f"dsc{t}")

            # phase A: h = gelu(x @ w1 + b1); dsc = gelu(x @ wd) * wte
            # k-outer / t-inner: consecutive matmuls share the stationary
            # weight tile, so the PE skips reloading it (load ~351 cyc)
            for m in range(MH + 1):
                if m < MH:
                    wk = w1_pool.tile([P, KC, P], F32R, tag="w1")
                    nc.sync.dma_start(
                        wk[:],
                        w1[:, m * P:(m + 1) * P].rearrange("(k p) m -> p k m", p=P))
                else:
                    wk = wd_sb
                pss = [psum_mm.tile([P, TT], F32, name=f"psa{t}", tag="ps") for t in tiles]
                for k in range(KC):
                    for ti, t in enumerate(tiles):
                        nc.tensor.matmul(pss[ti][:], wk[:, k, :],
                                         xts[t][:, k, :],
                                         start=(k == 0), stop=(k == KC - 1))
                for ti, t in enumerate(tiles):
                    if m < MH:
                        nc.scalar.activation(hs[t][:, m, :], pss[ti][:], ACT.Gelu,
                                             bias=b1_sb[:, m:m + 1])
                    else:
                        gd = small.tile([P, TT], F32, tag="gd", name=f"gd{t}")
                        nc.scalar.activation(gd[:], pss[ti][:], ACT.Gelu)
                        nc.vector.tensor_mul(dscs[t][:], gd[:],
                                             wte_sb[:, t * TT:(t + 1) * TT])

            # phase B: out = h @ w2 + b2 + dsc @ wu
            for m2 in range(MC):
                w2k = w2_pool.tile([P, MH, P], F32R, tag="w2")
                nc.sync.dma_start(
                    w2k[:],
                    w2[:, m2 * P:(m2 + 1) * P].rearrange("(k p) m -> p k m", p=P))
                pss = [psum_mm.tile([P, TT], F32, name=f"psb{t}", tag="ps") for t in tiles]
                for k in range(MH):
                    for ti, t in enumerate(tiles):
                        nc.tensor.matmul(pss[ti][:], w2k[:, k, :],
                                         hs[t][:, k, :],
                                         start=(k == 0), stop=False)
                for ti, t in enumerate(tiles):
                    nc.tensor.matmul(pss[ti][:], wu_sb[:, m2 * P:(m2 + 1) * P],
                                     dscs[t][:], start=False, stop=True)
                for ti, t in enumerate(tiles):
                    ob = out_pool.tile([P, TT], F32, tag="ob", name=f"ob{t}")
                    nc.scalar.activation(ob[:], pss[ti][:], ACT.Identity,
                                         bias=b2_sb[:, m2:m2 + 1])
                    nc.sync.dma_start(
                        outT[m2 * P:(m2 + 1) * P, t * TT:(t + 1) * TT], ob[:])

    nc.compile()
    return nc


_NC = None


def _get_nc():
    global _NC
    if _NC is None:
        _NC = _build()
    return _NC


def prepare_in_maps(x, topk_probs, topk_idx, w1, b1, w2, b2, w_down, w_up):
    x = np.asarray(x, dtype=np.float32)
    topk_probs = np.asarray(topk_probs, dtype=np.float32)
    topk_idx = np.asarray(topk_idx)
    w1 = np.ascontiguousarray(np.asarray(w1, dtype=np.float32))
    b1 = np.ascontiguousarray(np.asarray(b1, dtype=np.float32))
    w2 = np.ascontiguousarray(np.asarray(w2, dtype=np.float32))
    b2 = np.ascontiguousarray(np.asarray(b2, dtype=np.float32))
    w_down = np.asarray(w_down, dtype=np.float32)
    w_up = np.asarray(w_up, dtype=np.float32)

    xf = x.reshape(N, C)
    wd = np.ascontiguousarray(w_down.transpose(1, 0, 2).reshape(C, ER))
    wu = np.ascontiguousarray(w_up.reshape(ER, C))
    t_er = np.ascontiguousarray(
        np.broadcast_to((np.arange(P) // R).astype(np.float32), (P, P)))

    in_maps = []
    for i in range(NCORES):
        sl = slice(i * NL, (i + 1) * NL)
        in_maps.append({
            "xT": np.ascontiguousarray(xf[sl].T),
            "w1": w1, "b1": b1, "w2": w2, "b2": b2,
            "wd": wd, "wu": wu,
            "idxp": np.ascontiguousarray(np.concatenate(
                [topk_idx[sl].astype(np.float32),
                 topk_probs[sl].astype(np.float32)], axis=1)),
            "t_er": t_er,
        })
    return in_maps


def gather_out(results):
    out = np.concatenate([results[i]["outT"].T for i in range(NCORES)], axis=0)
    return np.ascontiguousarray(out.reshape(B, H, W, C))


def kernel(x, topk_probs, topk_idx, w1, b1, w2, b2, w_down, w_up):
    in_maps = prepare_in_maps(x, topk_probs, topk_idx, w1, b1, w2, b2,
                              w_down, w_up)
    res = run_bass_kernel_spmd(_get_nc(), in_maps, core_ids=list(range(NCORES)))
    return gather_out(res.results)
